# revision 1
# baseline (speedup 1.0000x reference)
"""Bass/Tile kernel for nn_EnergyDipolesMACE on 8 TRN2 NeuronCores (v3).

Host (index-only prep): drop edges with r >= R_MAX (their messages are
exactly zero: the polynomial cutoff zeroes rb and the radial MLP has no
biases), sort survivors by destination window, shard destination nodes
across cores (1024 each, 8 windows of 128), pad each window's edge list to
chunks of 128 (cap = max window load, uniform across cores for SPMD). Host
fancy-indexing supplies per-edge sndpos/rcvpos/h0[snd] and per-node
species-indexed weights, so the device only gathers data-dependent tensors
(iteration-2 h).

Device per core: geometry (Y, radial basis) once; radial MLP for BOTH
iterations fused (two 64-wide MLPs stacked on 128 partitions, block-diagonal
weights); one-hot scatter blocks built once on DVE and kept SBUF-resident;
per chunk: px matmul + message tensor-product on DVE + PE one-hot
scatter-matmul, with the node phase software-pipelined two windows behind;
AllGather of the updated scalar channel (optionally per-window) + SWDGE
re-gather feeds iteration 2.
"""
import math
import numpy as np

import concourse.bacc as bacc
import concourse.bass as bass
import concourse.tile as tile
from concourse import mybir

# allow 128B gather payloads (probed on HW previously)
import textwrap as _tw, inspect as _ins
try:
    _gsrc = _tw.dedent(_ins.getsource(bass.BassGpSimd.dma_gather))
except OSError:      # already patched by another module instance
    _gsrc = ""
if "% 256 == 0" in _gsrc:
    _gsrc = _gsrc.replace("elem_size_bytes > 0 and elem_size_bytes % 256 == 0",
                          "elem_size_bytes > 0 and elem_size_bytes % 128 == 0")
    _gns = dict(bass.__dict__)
    exec(compile(_gsrc, "<patched_dma_gather>", "exec"), _gns)
    bass.BassGpSimd.dma_gather = _gns["dma_gather"]

f32 = mybir.dt.float32
bf16 = mybir.dt.bfloat16
i16 = mybir.dt.int16
i32 = mybir.dt.int32
AF = mybir.ActivationFunctionType
ALU = mybir.AluOpType

N, E, C, Z, G, NB, NSH = 8192, 131072, 32, 10, 16, 8, 9
R_MAX, P_CUT, AVG_NEIGH = 5.0, 5, 16.0
LMAP = np.array([0, 1, 1, 1, 2, 2, 2, 2, 2])
NCORES = 8
NPC = N // NCORES
WIN = 128
WPC = NPC // WIN               # 8 windows/core
CHUNK = 128
NQ = 4                         # SWDGE queues
HROW = 64                      # agout row f32 elems (256B step; 128B payload)
MC = NSH * C                   # 288
S3, S5, S15 = 3.0 ** 0.5, 5.0 ** 0.5, 15.0 ** 0.5
PREF = (2.0 / R_MAX) ** 0.5
PCF = float(P_CUT)
ENV_A = -(PCF + 1.0) * (PCF + 2.0) / 2.0
ENV_B = PCF * (PCF + 2.0)
ENV_C = -PCF * (PCF + 1.0) / 2.0
TWO_PI = 2 * math.pi

DEFAULT_W_CAP = 11             # for reference.setup_inputs() (key(0)) inputs
_DYN = {"W_CAP": DEFAULT_W_CAP}


class Dims:
    def __init__(self, cap):
        self.W_CAP = cap
        self.L_PAD = WPC * cap * CHUNK
        self.NCHUNKS = WPC * cap
        self.IDX_COLS = self.L_PAD // 16
        # per-window chunk blocks (yh3/MLP granularity), uniform across windows
        self.BLOCKS = [3] * (cap // 3) + ([cap % 3] if cap % 3 else [])


def const_specs(D):
    return dict(
        iota=([128, 128], f32), rcvloc=([128, D.NCHUNKS], f32),
        ident=([128, 128], f32), nvec=([128, NB], f32),
        R0cat=([NB, 128], f32), R1bd=([128, 128], f32), R2bd=([128, 128], f32),
        R3z=([128, 2, MC], f32), Wmix=([C, 2, NSH, C], f32), Wsc=([C, 2, NSH, C], f32),
        Wro=([C, 19], f32), wE2=([16, 1], f32),
        e0own=([128, WPC], f32), h0oT=([C, WPC, 128], f32),
        goh=([128, WPC, G], f32), qown=([128, WPC], f32), posown=([128, WPC, 3], f32),
        w123=([128, WPC, 2, 3 * C], f32),
        gsnd=([128, D.IDX_COLS], i16), gsnd2=([128, D.IDX_COLS], i16),
    )


def big_input_specs(D):
    return dict(
        sndpos=([128, D.NCHUNKS, 3], f32), rcvpos=([128, D.NCHUNKS, 3], f32),
        hs0=([128, D.NCHUNKS, C], f32),
    )


def host_prep(inputs):
    snd = np.asarray(inputs["edge_index"])[0].astype(np.int64)
    rcv = np.asarray(inputs["edge_index"])[1].astype(np.int64)
    batch = np.asarray(inputs["batch"]).astype(np.int64)
    positions = np.asarray(inputs["positions"], np.float32)
    node_attrs = np.asarray(inputs["node_attrs"], np.float32)
    charges = np.asarray(inputs["charges"], np.float32)

    # exact sparsity: r >= R_MAX edges have identically-zero messages
    dvec = positions[rcv].astype(np.float64) - positions[snd].astype(np.float64)
    keep = (dvec * dvec).sum(1) < R_MAX * R_MAX * (1 + 1e-6)
    snd, rcv = snd[keep], rcv[keep]

    order = np.argsort(rcv, kind="stable")
    snd_s, rcv_s = snd[order], rcv[order]
    win_id = rcv_s // WIN
    counts = np.bincount(win_id, minlength=N // WIN)
    cap = max(1, -(-int(counts.max()) // CHUNK))
    _DYN["W_CAP"] = cap
    D = Dims(cap)

    iota = np.tile(np.arange(128, dtype=np.float32)[None, :], (128, 1))
    ident = np.eye(128, dtype=np.float32)
    nvec = np.tile((np.arange(1, NB + 1, dtype=np.float32) * math.pi / R_MAX)[None, :],
                   (128, 1))
    # MLP weights: both iterations stacked (64+64 features on 128 partitions)
    R0cat = np.concatenate([np.asarray(inputs["R0"][i], np.float32)
                            for i in range(2)], 1)            # [8, 128]
    R1bd = np.zeros((128, 128), np.float32)
    R2bd = np.zeros((128, 128), np.float32)
    for i in range(2):
        R1bd[i*64:(i+1)*64, i*64:(i+1)*64] = np.asarray(inputs["R1"][i], np.float32)
        R2bd[i*64:(i+1)*64, i*64:(i+1)*64] = np.asarray(inputs["R2"][i], np.float32)
    R3z = np.zeros((128, 2, MC), np.float32)
    for i in range(2):
        R3z[i*64:(i+1)*64, i, :] = (np.asarray(inputs["R3"][i], np.float32)
                                    .reshape(64, 3, C)[:, LMAP, :].reshape(64, MC))
    Wmix = np.stack([np.asarray(inputs["W_mix"][i], np.float32)[LMAP] for i in range(2)], 0)
    Wmix = Wmix.transpose(2, 0, 1, 3).copy()                  # [C, 2, 9, C]
    Wsc = np.stack([np.asarray(inputs["W_sc"][i], np.float32)[LMAP] for i in range(2)], 0)
    Wsc = Wsc.transpose(2, 0, 1, 3).copy()
    Wro = np.concatenate([np.asarray(inputs["wE1"], np.float32)[:, None],
                          np.asarray(inputs["wD1"], np.float32)[:, None],
                          np.asarray(inputs["Wh"], np.float32),
                          np.asarray(inputs["wD2"], np.float32)[:, None]], 1)
    wE2 = np.asarray(inputs["wE2"], np.float32)[:, None]

    # host index-prep: per-node species lookups (node_attrs is one-hot)
    h0full = node_attrs @ np.asarray(inputs["W_embed"], np.float32)   # [N, C]
    e0full = node_attrs @ np.asarray(inputs["atomic_energies"], np.float32)  # [N]
    wfull = [[node_attrs @ np.asarray(inputs[f"Wp{j}"], np.float32)[i]
              for j in (1, 2, 3)] for i in range(2)]          # [2][3] of [N, C]

    shared = dict(iota=iota, ident=ident, nvec=nvec, R0cat=R0cat, R1bd=R1bd,
                  R2bd=R2bd, R3z=R3z, Wmix=Wmix, Wsc=Wsc, Wro=Wro, wE2=wE2)

    in_maps = []
    for k in range(NCORES):
        snd_pad = np.zeros(D.L_PAD, np.int64)
        rcv_glob = np.zeros(D.L_PAD, np.int64)
        rcv_loc = np.full(D.L_PAD, -1000.0, np.float32)
        for w in range(WPC):
            gw = k * WPC + w
            sel = win_id == gw
            cnt = int(counts[gw])
            base = w * cap * CHUNK
            snd_pad[base:base + cnt] = snd_s[sel]
            rcv_glob[base:base + cnt] = rcv_s[sel]
            rcv_loc[base:base + cnt] = (rcv_s[sel] - gw * WIN).astype(np.float32)

        def wrap_idx(a):
            w16 = a.astype(np.int16).reshape(D.IDX_COLS, 16).T
            return np.tile(w16, (8, 1)).copy()

        def edge_fmt(a):  # [L_PAD, d] -> [128, NCHUNKS, d]
            d = a.shape[1]
            return np.ascontiguousarray(
                a.reshape(D.NCHUNKS, CHUNK, d).transpose(1, 0, 2))

        own = slice(k * NPC, (k + 1) * NPC)
        m = dict(shared)
        m["gsnd"] = wrap_idx(snd_pad)
        m["gsnd2"] = wrap_idx((snd_pad % NPC) // WIN * NPC
                              + (snd_pad // NPC) * WIN + (snd_pad % WIN))
        m["rcvloc"] = np.ascontiguousarray(rcv_loc.reshape(D.NCHUNKS, CHUNK).T)
        m["sndpos"] = edge_fmt(positions[snd_pad])
        m["rcvpos"] = edge_fmt(positions[rcv_glob])
        m["hs0"] = edge_fmt(h0full[snd_pad])
        m["e0own"] = np.ascontiguousarray(e0full[own].reshape(WPC, 128).T)
        m["w123"] = np.ascontiguousarray(
            np.stack([np.concatenate([wfull[i][j][own] for j in range(3)], 1)
                      for i in range(2)], 1)                  # [NPC, 2, 3C]
            .reshape(WPC, 128, 2, 3 * C).transpose(1, 0, 2, 3))
        m["h0oT"] = np.ascontiguousarray(
            h0full[own].reshape(WPC, 128, C).transpose(2, 0, 1))  # [C, WPC, 128]
        goh = np.zeros((NPC, G), np.float32)
        goh[np.arange(NPC), batch[own]] = 1.0
        m["goh"] = np.ascontiguousarray(goh.reshape(WPC, 128, G).transpose(1, 0, 2))
        m["qown"] = np.ascontiguousarray(charges[own].reshape(WPC, 128).T)
        m["posown"] = np.ascontiguousarray(
            positions[own].reshape(WPC, 128, 3).transpose(1, 0, 2))
        in_maps.append(m)
    return in_maps, {}


def build_nc(num_devices=NCORES, sim_safe=False, phases=99, repeat=1, agmode=0,
             w_cap=None, msg2x=0, wkbufs=3, **_kw):
    D = Dims(w_cap if w_cap is not None else _DYN["W_CAP"])
    CSPEC, BSPEC = const_specs(D), big_input_specs(D)
    nc = bacc.Bacc("TRN2", target_bir_lowering=False, debug=False,
                   num_devices=num_devices, num_swdge_queues=NQ)
    inp = {name: nc.dram_tensor(name, shape, dt, kind="ExternalInput")
           for name, (shape, dt) in {**CSPEC, **BSPEC}.items()}
    y_out = nc.dram_tensor("y", [G, 4], f32, kind="ExternalOutput")
    agin = nc.dram_tensor("agin", [NPC, HROW], f32, kind="Internal")
    agout = nc.dram_tensor("agout", [N, HROW], f32, kind="Internal",
                           addr_space="Shared")

    def silu(out_ap, in_ap, pool, tag="siltmp"):
        if not sim_safe:
            nc.scalar.activation(out_ap, in_ap, AF.Silu)
        else:
            sg = pool.tile(list(out_ap.shape), f32, tag=tag)
            nc.scalar.activation(sg[:], in_ap, AF.Sigmoid)
            nc.vector.tensor_tensor(out_ap, in_ap, sg[:], ALU.mult)

    GCH = 6                        # chunks per gather call (768-idx ring limit)

    def gather_h(dst_tile, src_dram, idx_tile):
        # call j covers chunks [j*6, j*6+6), queue j%NQ: consecutive windows
        # land on different queues so window w's chunks are ready ~in order
        for j in range(-(-D.NCHUNKS // GCH)):
            b = j * GCH
            g = min(GCH, D.NCHUNKS - b)
            nc.gpsimd.dma_gather(
                out_ap=dst_tile[:, b:b + g, :],
                in_ap=src_dram.ap()[:, 0:C],
                idxs_ap=idx_tile[:, b * 8:(b + g) * 8],
                num_idxs=g * CHUNK, num_idxs_reg=g * CHUNK,
                elem_size=C, elem_step=HROW, queue_num=j % NQ)

    with tile.TileContext(nc) as tc:
        with tc.tile_pool(name="const", bufs=1) as cst, \
             tc.tile_pool(name="big", bufs=1) as big, \
             tc.tile_pool(name="pmlp", bufs=1, space="PSUM") as pmlp, \
             tc.tile_pool(name="px", bufs=2, space="PSUM") as pxp, \
             tc.tile_pool(name="pa", bufs=3, space="PSUM") as pap, \
             tc.tile_pool(name="pmisc", bufs=2, space="PSUM") as pms:

            sb = {}
            for name, (shape, dt) in CSPEC.items():
                t = cst.tile(shape, dt, tag=f"c_{name}")
                nc.sync.dma_start(out=t[:], in_=inp[name].ap())
                sb[name] = t
            # bf16 weight copies
            R0b = cst.tile([NB, 128], bf16, tag="R0b")
            nc.scalar.activation(R0b[:], sb["R0cat"][:], AF.Copy)
            R1b = cst.tile([128, 128], bf16, tag="R1b")
            nc.scalar.activation(R1b[:], sb["R1bd"][:], AF.Copy)
            R2b = cst.tile([128, 128], bf16, tag="R2b")
            nc.scalar.activation(R2b[:], sb["R2bd"][:], AF.Copy)
            R3zb = cst.tile([128, 2, MC], bf16, tag="R3zb")
            nc.scalar.activation(R3zb[:], sb["R3z"][:], AF.Copy)
            Wscb = cst.tile([C, 2, NSH, C], bf16, tag="Wscb")
            nc.scalar.activation(Wscb[:], sb["Wsc"][:], AF.Copy)
            Wrob = cst.tile([C, 19], bf16, tag="Wrob")
            nc.scalar.activation(Wrob[:], sb["Wro"][:], AF.Copy)
            wE2b = cst.tile([16, 1], bf16, tag="wE2b")
            nc.scalar.activation(wE2b[:], sb["wE2"][:], AF.Copy)
            h0oTb = cst.tile([C, WPC, 128], bf16, tag="h0oTb")
            nc.scalar.activation(h0oTb[:], sb["h0oT"][:], AF.Copy)

            # persistent tiles
            hsE = big.tile([128, D.NCHUNKS, C], f32, tag="hsE")
            Ysb = big.tile([128, D.NCHUNKS, NSH], f32, tag="Y")
            s3_all = big.tile([128, D.L_PAD], bf16, tag="s3_all")
            ohsb = big.tile([128, D.NCHUNKS, 128], bf16, tag="ohsb")
            hT = big.tile([C, WPC, NSH * 128], bf16, tag="hT")
            e0_sb = sb["e0own"]
            vals = big.tile([128, WPC, 4], f32, tag="vals")
            rbw = big.tile([128, D.NCHUNKS, NB], f32, tag="rbw")

            for _rep in range(repeat):
              # ---- geometry (scratch scope)
              if phases >= 1:
                with tc.tile_pool(name="geos", bufs=1) as gsc:
                    spos = gsc.tile([128, D.NCHUNKS, 3], f32, tag="spos")
                    nc.sync.dma_start(out=spos[:], in_=inp["sndpos"].ap())
                    rpos = gsc.tile([128, D.NCHUNKS, 3], f32, tag="rpos")
                    nc.sync.dma_start(out=rpos[:], in_=inp["rcvpos"].ap())
                    nc.sync.dma_start(out=hsE[:], in_=inp["hs0"].ap())

                    geo = gsc.tile([128, D.NCHUNKS, 14], f32, tag="geo")
                    vec, sq = geo[:, :, 0:3], geo[:, :, 3:6]
                    r2, r_, rinv = geo[:, :, 6], geo[:, :, 7], geo[:, :, 8]
                    u = geo[:, :, 9:12]
                    t0, t1 = geo[:, :, 12], geo[:, :, 13]
                    BC = [128, D.NCHUNKS, 3]
                    nc.vector.tensor_tensor(vec, rpos[:], spos[:], ALU.subtract)
                    nc.scalar.square(sq, vec)
                    nc.vector.tensor_reduce(r2.unsqueeze(2), sq, mybir.AxisListType.X, ALU.add)
                    nc.vector.tensor_scalar_add(r2.unsqueeze(2), r2.unsqueeze(2), 1e-12)
                    nc.scalar.activation(r_.unsqueeze(2), r2.unsqueeze(2), AF.Sqrt)
                    nc.vector.reciprocal(rinv.unsqueeze(2), r_.unsqueeze(2))
                    nc.vector.tensor_tensor(u, vec, rinv.unsqueeze(2).broadcast_to(BC),
                                            ALU.mult)
                    ux = u[:, :, 0].unsqueeze(2)
                    uy = u[:, :, 1].unsqueeze(2)
                    uz = u[:, :, 2].unsqueeze(2)
                    nc.vector.memset(Ysb[:, :, 0].unsqueeze(2), 1.0)
                    nc.scalar.activation(Ysb[:, :, 1:4], u, AF.Copy, scale=S3)
                    nc.vector.scalar_tensor_tensor(Ysb[:, :, 4].unsqueeze(2), ux, S15, uy,
                                                   ALU.mult, ALU.mult)
                    nc.vector.scalar_tensor_tensor(Ysb[:, :, 5].unsqueeze(2), uy, S15, uz,
                                                   ALU.mult, ALU.mult)
                    nc.vector.tensor_tensor(t0.unsqueeze(2), uz, uz, ALU.mult)
                    nc.scalar.activation(Ysb[:, :, 6].unsqueeze(2), t0.unsqueeze(2), AF.Copy,
                                         scale=3.0 * S5 / 2.0, bias=-S5 / 2.0)
                    nc.vector.scalar_tensor_tensor(Ysb[:, :, 7].unsqueeze(2), ux, S15, uz,
                                                   ALU.mult, ALU.mult)
                    nc.vector.tensor_tensor(t0.unsqueeze(2), ux, uy, ALU.add)
                    nc.vector.tensor_tensor(t1.unsqueeze(2), ux, uy, ALU.subtract)
                    nc.vector.scalar_tensor_tensor(Ysb[:, :, 8].unsqueeze(2),
                                                   t0.unsqueeze(2), S15 / 2.0,
                                                   t1.unsqueeze(2), ALU.mult, ALU.mult)
                    # radial basis arg
                    BC8 = [128, D.NCHUNKS, NB]
                    nc.vector.tensor_tensor(rbw[:], r_.unsqueeze(2).broadcast_to(BC8),
                                            sb["nvec"].unsqueeze(1).broadcast_to(BC8),
                                            ALU.mult)
                    # range-reduce to [-pi, pi]
                    rmsk = gsc.tile([128, D.NCHUNKS, NB], f32, tag="rmsk")
                    rki = gsc.tile([128, D.NCHUNKS, NB], i32, tag="rki")
                    nc.scalar.activation(rmsk[:], rbw[:], AF.Copy, scale=1.0 / TWO_PI)
                    nc.vector.tensor_copy(rki[:], rmsk[:])
                    nc.vector.tensor_copy(rmsk[:], rki[:])
                    nc.vector.scalar_tensor_tensor(rbw[:], rmsk[:], -TWO_PI, rbw[:],
                                                   ALU.mult, ALU.add)
                    nc.vector.tensor_scalar(rmsk[:], rbw[:], math.pi, None, ALU.is_gt)
                    nc.vector.scalar_tensor_tensor(rbw[:], rmsk[:], -TWO_PI, rbw[:],
                                                   ALU.mult, ALU.add)
                    nc.vector.tensor_scalar(rbw[:], rbw[:], math.pi, None, ALU.min)
                    nc.vector.tensor_scalar(rbw[:], rbw[:], -math.pi, None, ALU.max)
                    nc.scalar.activation(rbw[:], rbw[:], AF.Sin)
                    # envelope
                    xx = t0.unsqueeze(2)
                    nc.vector.tensor_scalar(xx, r_.unsqueeze(2), 1.0 / R_MAX, None, ALU.mult)
                    x2 = t1.unsqueeze(2)
                    nc.scalar.square(x2, xx)
                    x4 = geo[:, :, 3].unsqueeze(2)
                    nc.scalar.square(x4, x2)
                    x5 = geo[:, :, 4].unsqueeze(2)
                    nc.vector.tensor_tensor(x5, x4, xx, ALU.mult)
                    q1 = geo[:, :, 5].unsqueeze(2)
                    nc.scalar.activation(q1, xx, AF.Copy, scale=ENV_C, bias=ENV_B)
                    q2 = t1.unsqueeze(2)
                    nc.vector.tensor_tensor(q2, q1, xx, ALU.mult)
                    nc.vector.tensor_scalar_add(q2, q2, ENV_A)
                    env = r2.unsqueeze(2)
                    nc.vector.tensor_tensor(env, x5, q2, ALU.mult)
                    nc.vector.tensor_scalar_add(env, env, 1.0)
                    mlt = geo[:, :, 3].unsqueeze(2)
                    nc.vector.tensor_scalar(mlt, xx, 1.0, None, ALU.is_lt)
                    nc.vector.tensor_tensor(env, env, mlt, ALU.mult)
                    wfac = geo[:, :, 4].unsqueeze(2)
                    nc.vector.scalar_tensor_tensor(wfac, rinv.unsqueeze(2), PREF, env,
                                                   ALU.mult, ALU.mult)
                    nc.vector.tensor_tensor(rbw[:], rbw[:],
                                            wfac.broadcast_to(BC8), ALU.mult)

                # one-hot blocks, SBUF-resident (DVE; overlaps MLP Act/PE work)
                for ch in range(D.NCHUNKS):
                    nc.vector.tensor_scalar(
                        ohsb[:, ch, :], sb["iota"][:],
                        sb["rcvloc"][:, ch].unsqueeze(1),
                        1.0 / AVG_NEIGH, ALU.is_equal, ALU.mult)

              # ---- iterations (it=0 also runs the fused both-iteration MLP
              # per chunk-block, so Act/PE overlap DVE msg work)
              with tc.tile_pool(name="wk", bufs=wkbufs) as wk, \
                   tc.tile_pool(name="nd", bufs=2) as ndp:
                  nc.vector.memset(vals[:], 0.0)

                  def emit_msg_phase(it, w):
                      pA = pap.tile([128, MC], f32, tag="pA")
                      nblk = len(D.BLOCKS)
                      w0 = w * D.W_CAP
                      two_x = msg2x and it == 1
                      yh3 = None
                      sp0 = spn = 0
                      g3 = w0
                      for bi, blk in enumerate(D.BLOCKS):
                          ee = g3 * CHUNK
                          if g3 + blk > w0 + sp0 + spn:
                              # next yh3 span (up to 6 chunks)
                              sp0 = g3 - w0
                              spn = min(6, D.W_CAP - sp0)
                              yh3 = wk.tile([128, 6, MC], bf16 if two_x else f32,
                                            tag="yh3b" if two_x else "yh3")
                              nc.vector.tensor_tensor(
                                  yh3[:, 0:spn, :].rearrange("p t (m c) -> p t m c", m=NSH),
                                  Ysb[:, w0 + sp0:w0 + sp0 + spn, :].unsqueeze(3)
                                      .broadcast_to([128, spn, NSH, C]),
                                  hsE[:, w0 + sp0:w0 + sp0 + spn, :].unsqueeze(2)
                                      .broadcast_to([128, spn, NSH, C]),
                                  ALU.mult)
                          if it == 0:
                              ptr = pms.tile([NB, 3 * CHUNK], f32, tag="pm")
                              for j in range(blk):
                                  nc.tensor.transpose(ptr[:, j * 128:(j + 1) * 128],
                                                      rbw[:, g3 + j, :], sb["ident"][:])
                              rbTs = wk.tile([NB, 3 * CHUNK], bf16, tag="rbTs")
                              nc.scalar.activation(rbTs[:, 0:blk * CHUNK],
                                                   ptr[:, 0:blk * CHUNK], AF.Copy)
                              p1 = pmlp.tile([128, 3 * CHUNK], f32, tag="pmlp")
                              nc.tensor.matmul(p1[:, 0:blk * CHUNK], R0b[:],
                                               rbTs[:, 0:blk * CHUNK],
                                               start=True, stop=True)
                              s1 = wk.tile([128, 3 * CHUNK], bf16, tag="s1")
                              silu(s1[:, 0:blk * CHUNK], p1[:, 0:blk * CHUNK], wk)
                              p2 = pmlp.tile([128, 3 * CHUNK], f32, tag="pmlp")
                              nc.tensor.matmul(p2[:, 0:blk * CHUNK], R1b[:],
                                               s1[:, 0:blk * CHUNK], start=True, stop=True)
                              s2 = wk.tile([128, 3 * CHUNK], bf16, tag="s2")
                              silu(s2[:, 0:blk * CHUNK], p2[:, 0:blk * CHUNK], wk)
                              p3 = pmlp.tile([128, 3 * CHUNK], f32, tag="pmlp")
                              nc.tensor.matmul(p3[:, 0:blk * CHUNK], R2b[:],
                                               s2[:, 0:blk * CHUNK], start=True, stop=True)
                              silu(s3_all[:, ee:ee + blk * CHUNK],
                                   p3[:, 0:blk * CHUNK], wk)
                          for j in range(blk):
                              ch = g3 + j
                              px = pxp.tile([128, MC], f32, tag="px")
                              nc.tensor.matmul(px[:],
                                               s3_all[:, ch * 128:(ch + 1) * 128],
                                               R3zb[:, it, :],
                                               start=True, stop=True)
                              msg = wk.tile([128, MC], bf16, tag="msg")
                              if two_x:
                                  pxb = wk.tile([128, MC], bf16, tag="pxb")
                                  nc.scalar.activation(pxb[:], px[:], AF.Copy)
                                  nc.vector.tensor_tensor(msg[:], yh3[:, ch - w0 - sp0, :],
                                                          pxb[:], ALU.mult)
                              else:
                                  nc.vector.tensor_tensor(msg[:], yh3[:, ch - w0 - sp0, :],
                                                          px[:], ALU.mult)
                              nc.tensor.matmul(pA[:], ohsb[:, ch, :], msg[:],
                                               start=(bi == 0 and j == 0),
                                               stop=(bi == nblk - 1 and j == blk - 1))
                          g3 += blk
                      return pA

                  def emit_node_phase(it, w, pA):
                      if True:
                          A_sb = ndp.tile([128, MC], f32, tag="Asb")
                          nc.scalar.activation(A_sb[:], pA[:], AF.Copy)
                          AT = ndp.tile([C, NSH * 128], f32, tag="AT")
                          for t4 in range(3):
                              hi = min(4, NSH - t4 * 4)
                              ptA = pms.tile([C, 512], f32, tag="pm")
                              for j in range(hi):
                                  mm = t4 * 4 + j
                                  nc.tensor.transpose(ptA[:, j * 128:(j + 1) * 128],
                                                      A_sb[:, mm * C:(mm + 1) * C],
                                                      sb["ident"][:])
                              nc.scalar.activation(AT[:, t4 * 512:t4 * 512 + hi * 128],
                                                   ptA[:, 0:hi * 128], AF.Copy)
                          pA2 = pms.tile([128, MC], f32, tag="pm")
                          for mm in range(NSH):
                              nc.tensor.matmul(pA2[:, mm * C:(mm + 1) * C],
                                               AT[:, mm * 128:(mm + 1) * 128],
                                               sb["Wmix"][:, it, mm, :],
                                               start=True, stop=True)
                          psc = pms.tile([128, MC], f32, tag="pm")
                          if it == 0:
                              nc.tensor.matmul(psc[:, 0:C], h0oTb[:, w, :],
                                               Wscb[:, 0, 0, :], start=True, stop=True)
                              sc_sb = ndp.tile([128, C], f32, tag="scsb")
                              nc.scalar.activation(sc_sb[:], psc[:, 0:C], AF.Copy)
                          else:
                              for mm in range(NSH):
                                  nc.tensor.matmul(psc[:, mm * C:(mm + 1) * C],
                                                   hT[:, w, mm * 128:(mm + 1) * 128],
                                                   Wscb[:, 1, mm, :],
                                                   start=True, stop=True)
                              sc_sb = ndp.tile([128, MC], f32, tag="scsb9")
                              nc.scalar.activation(sc_sb[:], psc[:], AF.Copy)
                          wslc = sb["w123"][:, w, it, :]
                          F = ndp.tile([128, C], f32, tag="F")
                          nc.vector.tensor_tensor(F[:], wslc[:, 2 * C:3 * C],
                                                  pA2[:, 0:C], ALU.mult)
                          nc.vector.tensor_tensor(F[:], F[:], wslc[:, C:2 * C], ALU.add)
                          nc.vector.tensor_tensor(F[:], F[:], pA2[:, 0:C], ALU.mult)
                          nc.vector.tensor_tensor(F[:], F[:], wslc[:, 0:C], ALU.add)
                          hw_t = ndp.tile([128, MC], f32, tag="hw")
                          nc.vector.tensor_tensor(
                              hw_t[:].rearrange("p (m c) -> p m c", m=NSH),
                              pA2[:].rearrange("p (m c) -> p m c", m=NSH),
                              F[:].unsqueeze(1).broadcast_to([128, NSH, C]), ALU.mult)
                          if it == 0:
                              nc.vector.tensor_tensor(hw_t[:, 0:C], hw_t[:, 0:C],
                                                      sc_sb[:], ALU.add)
                          else:
                              nc.vector.tensor_tensor(hw_t[:], hw_t[:], sc_sb[:], ALU.add)
                          n_m = NSH if it == 0 else 4
                          for t4 in range((n_m + 3) // 4):
                              hi = min(4, n_m - t4 * 4)
                              pth = pms.tile([C, 512], f32, tag="pm")
                              for j in range(hi):
                                  mm = t4 * 4 + j
                                  nc.tensor.transpose(pth[:, j * 128:(j + 1) * 128],
                                                      hw_t[:, mm * C:(mm + 1) * C],
                                                      sb["ident"][:])
                              nc.scalar.activation(
                                  hT[:, w, t4 * 512:t4 * 512 + hi * 128],
                                  pth[:, 0:hi * 128], AF.Copy)
                          if it == 0:
                              nc.sync.dma_start(
                                  out=agin.ap()[w * 128:(w + 1) * 128, 0:C],
                                  in_=hw_t[:, 0:C])
                              prd = pms.tile([128, 4], f32, tag="pm")
                              nc.tensor.matmul(prd[:, 0:1], hT[:, w, 0:128], Wrob[:, 0:1],
                                               start=True, stop=True)
                              for mm in (1, 2, 3):
                                  nc.tensor.matmul(prd[:, mm:mm + 1],
                                                   hT[:, w, mm * 128:(mm + 1) * 128],
                                                   Wrob[:, 1:2], start=True, stop=True)
                              nc.vector.scalar_tensor_tensor(
                                  vals[:, w, 0].unsqueeze(1), prd[:, 0:1], 1.0,
                                  e0_sb[:, w].unsqueeze(1), ALU.mult, ALU.add)
                              nc.scalar.activation(vals[:, w, 1:4], prd[:, 1:4], AF.Copy)
                          else:
                              phid = pms.tile([128, 16], f32, tag="pm")
                              nc.tensor.matmul(phid[:], hT[:, w, 0:128], Wrob[:, 2:18],
                                               start=True, stop=True)
                              hid = ndp.tile([128, 16], f32, tag="hid")
                              silu(hid[:], phid[:], ndp)
                              pht = pms.tile([16, 128], f32, tag="pm")
                              nc.tensor.transpose(pht[:], hid[:], sb["ident"][:])
                              hidT = ndp.tile([16, 128], bf16, tag="hidT")
                              nc.scalar.activation(hidT[:], pht[:], AF.Copy)
                              prd = pms.tile([128, 4], f32, tag="pm")
                              nc.tensor.matmul(prd[:, 0:1], hidT[:], wE2b[:],
                                               start=True, stop=True)
                              for mm in (1, 2, 3):
                                  nc.tensor.matmul(prd[:, mm:mm + 1],
                                                   hT[:, w, mm * 128:(mm + 1) * 128],
                                                   Wrob[:, 18:19], start=True, stop=True)
                              nc.vector.tensor_tensor(vals[:, w, :], vals[:, w, :],
                                                      prd[:], ALU.add)
                              nc.vector.scalar_tensor_tensor(
                                  vals[:, w, 1:4], sb["posown"][:, w, :],
                                  sb["qown"][:, w].unsqueeze(1), vals[:, w, 1:4],
                                  ALU.mult, ALU.add)

                  def post_node(it, w):
                      # per-window AllGather (agmode=1): hide all but the
                      # last collective behind remaining it=0 compute
                      if it == 0 and phases >= 3 and num_devices > 1 and agmode == 1:
                          nc.gpsimd.collective_compute(
                              "AllGather", ALU.bypass,
                              replica_groups=[list(range(num_devices))],
                              ins=[agin.ap()[w * WIN:(w + 1) * WIN, :]],
                              outs=[agout.ap()[w * NPC:(w + 1) * NPC, :]])

                  STAG = 2
                  for it in range(2 if phases >= 3 else (1 if phases >= 2 else 0)):
                      pend = {}
                      for w in range(WPC):
                          pend[w] = emit_msg_phase(it, w)
                          if w >= STAG:
                              emit_node_phase(it, w - STAG, pend.pop(w - STAG))
                              post_node(it, w - STAG)
                      for w in range(WPC - STAG, WPC):
                          emit_node_phase(it, w, pend.pop(w))
                          post_node(it, w)

                      if it == 0 and phases >= 3:
                          if num_devices > 1:
                              if agmode == 0:
                                  nc.gpsimd.collective_compute(
                                      "AllGather", ALU.bypass,
                                      replica_groups=[list(range(num_devices))],
                                      ins=[agin.ap()], outs=[agout.ap()])
                                  gather_h(hsE, agout, sb["gsnd"])
                              else:
                                  gather_h(hsE, agout, sb["gsnd2"])
                          else:
                              nc.sync.dma_start(out=hsE[:], in_=inp["hs0"].ap())

                  # final reduction
                  pO = pms.tile([G, 4], f32, tag="pm")
                  if phases < 3:
                      for w in range(WPC):
                          nc.vector.scalar_tensor_tensor(
                              vals[:, w, 1:4], sb["posown"][:, w, :],
                              sb["qown"][:, w].unsqueeze(1), vals[:, w, 1:4],
                              ALU.mult, ALU.add)
                  for w in range(WPC):
                      nc.tensor.matmul(pO[:], sb["goh"][:, w, :], vals[:, w, :],
                                       start=(w == 0), stop=(w == WPC - 1))
                  y_sb = ndp.tile([G, 4], f32, tag="ysb")
                  nc.scalar.activation(y_sb[:], pO[:], AF.Copy)
                  nc.sync.dma_start(out=y_out.ap(), in_=y_sb[:])

    nc.compile()
    return nc


from concourse.bass_utils import run_bass_kernel_spmd as _run_spmd

_NC_CACHE = {}


def _get_nc():
    key = ("nc", _DYN["W_CAP"])
    if key not in _NC_CACHE:
        _NC_CACHE[key] = build_nc(num_devices=NCORES, sim_safe=False)
    return _NC_CACHE[key]


def kernel(**inputs):
    np_inputs = {k: np.asarray(v) for k, v in inputs.items()}
    in_maps, _ = host_prep(np_inputs)
    nc = _get_nc()
    res = _run_spmd(nc, in_maps, core_ids=list(range(NCORES)))
    y = sum(np.asarray(res.results[k]["y"], dtype=np.float64)
            for k in range(NCORES))
    return y.astype(np.float32)

